# revision 35
# baseline (speedup 1.0000x reference)
"""Trainium2 Bass kernel for CDimSelfAttention (v2).

Problem: x [B=4, K=8, T=2048, C=64] f32; per (b,k) head:
  q = x @ Wq.T + bq ; k = x @ Wk.T + bk ; v = x @ Wv.T + bv
  out = softmax(q k^T / sqrt(C)) v

Sharding: data-parallel over flattened (b,k) — 32 heads, 4 per core on
8 cores. Weights replicated.

v2 design (vs v1 baseline at ~249us): instruction-count and engine-balance
rework, keeping v1's permuted-column trick (column u = g*128 + p <-> row
t = 16p + g, softmax/AV permutation-invariant, un-permuted for free at the
final store).

  - x [128,1024] f32 -> fp16 once (DVE), fp16 PE transposes (1 cyc/col),
    batched partition-shifted DVE copies into xT_aug [65, 2048] whose row 64
    is constant 1.0 (ones-row).
  - Biases folded into the matmuls via the ones-row: one stationary
    wqk_aug [65, 128] computes q (psum rows 0:64) AND k (rows 64:128) with
    biases, in 4x 512-col matmuls -> one fp16 copy -> qkT; kT extracted by a
    cheap 4x-mode SBUF fp16 copy (partition shift).
  - v~ [128, 16*65] fp16 via 16 matmuls vs wv_aug [65, 65] (bias row + ones
    column folded), 4 batched psum->fp16 converts.
  - exp(s - 1.7) (global shift, cancels in softmax; keeps exp < fp16 range):
    ACT activation for 13 of 16 j-tiles; DVE computes the other 3 via a
    one-op Schraudolph: uint16 bits = s*(log2e*128) + const -> bitcast fp16.
    (measured HW: total rel err 8.3e-3 vs 2e-2 budget)
  - AV accumulated in psum [65, 1024] per i-half (v1 scheme), ones column
    gives row sums.
  - Finalize: psav -> fp16 avs, fp16 PE transposes into one psum tile
    [128, 8*80], one strided reciprocal (row sums), one broadcast
    tensor_mul -> out_sb; single DMA store per head.

Software pipelining: the (ihalf, j) chunk loop is flattened so S/exp
pipeline across the i-half boundary (only AV waits on the psav barrier,
absorbed by the AV_LAG emission lag); phase 1 of head h+1 is interleaved
into head h's chunk loop as generator steps, and head 0 starts its own
loop as soon as the first half of its phase 1 is emitted.

Measured on HW: v1 baseline 249.5us -> v2 240.5us. Both runs are bound by
the PE instruction stream (~1363 PE instructions x ~175ns dispatch/sem
overhead each); engine busy times (sim): PE 119us, ACT 109us, DVE 73us.
The matmul count is pinned by the PSUM-bank limit (a single matmul output
cannot cross a 2KB bank boundary -> max 512 fp32 columns), so S and AV
need 2 matmuls per [128, 1024] tile each, x32 tiles x4 heads, plus one
implicit Ldweights per matmul emitted by the tile scheduler.
"""

from collections import deque

import numpy as np

import concourse.bass as bass
import concourse.mybir as mybir
import concourse.tile as tile
from concourse import bacc
from concourse.bass_utils import run_bass_kernel_spmd
from concourse.masks import make_identity

F32 = mybir.dt.float32
F16 = mybir.dt.float16
U16 = mybir.dt.uint16
AF = mybir.ActivationFunctionType
ALU = mybir.AluOpType

B, K, T, C = 4, 8, 2048, 64
N_CORES = 8
HEADS = B * K            # 32
HPC = HEADS // N_CORES   # 4 heads per core
P = 128                  # partitions
NT = T // P              # 16 t-tiles / j-tiles
FREE = T * C // P        # 1024 free elems of one head slice on 128 partitions
C1 = C + 1               # 65: v plus ones column
IH = T // 2              # 1024, i-half size
REPEAT = 1               # repeat whole per-core workload (timing experiments)
SKIP_EXP = False
SKIP_AV = False
SKIP_S = False
SKIP_P1 = False
AV_LAG = 4               # AV matmul emission lag (steps)
AVT_DMA = False          # finalize transposes via DMA xbar instead of PE
ET_BUFS = 6
EXP_SHIFT = 1.7          # exp(s - shift): keeps exp(s) <= e^5.6 ~ 270
DVE_JS = (1, 6, 11)      # j-tiles whose exp runs on DVE (Schraudolph)
LOG2E = 1.4426950408889634
SCH_MAGIC = -59.0        # Schraudolph centering (fp16-bits units)
# uint16 fp16-bits = s_raw * SCH_A + SCH_B  (s_raw = q.k before /8 scaling)
SCH_A = 0.125 * LOG2E * 1024.0
SCH_B = 1024.0 * (15.0 - EXP_SHIFT * LOG2E) + SCH_MAGIC + 0.5  # +0.5: trunc->round


def _build_tile_kernel(tc, nc, x_d, wq_d, bq_d, wk_d, bk_d, wv_d, bv_d, out_d):
    ctxs = []

    def pool(**kw):
        cm = tc.tile_pool(**kw)
        p = cm.__enter__()
        ctxs.append(cm)
        return p

    try:
        consts = pool(name="consts", bufs=1)
        sb2 = pool(name="sb2", bufs=2)
        etp = pool(name="etp", bufs=ET_BUFS)
        # PSUM: 8 banks of 2KB/partition.
        #   s_pool: S^T matmul tiles [128,1024]f32 = 2 banks x 2 bufs = 4
        #   psav:   A@V accumulator  [65,1024]f32  = 2 banks x 1 buf  = 2
        #   psw:    work tiles (<=1 bank each)     = 1 bank x 2 bufs  = 2
        s_pool = pool(name="s_pool", bufs=2, space="PSUM")
        psw = pool(name="psw", bufs=2, space="PSUM")
        psav = pool(name="psav", bufs=1, space="PSUM")

        # ---- constants ----
        ident = consts.tile([P, P], F32)
        make_identity(nc, ident)
        ident16 = consts.tile([P, P], F16)
        nc.vector.tensor_copy(out=ident16, in_=ident)
        nbias = consts.tile([P, 1], F32)
        nc.gpsimd.memset(nbias, -EXP_SHIFT)

        # wqk_aug [65, 128] fp16: rows 0:64 = Wq^T | Wk^T, row 64 = bq | bk
        wstage = consts.tile([C1, P], F32)
        wq_n = consts.tile([C, C], F32)
        nc.sync.dma_start(out=wq_n, in_=wq_d.ap())
        wk_n = consts.tile([C, C], F32)
        nc.sync.dma_start(out=wk_n, in_=wk_d.ap())
        wv_n = consts.tile([C, C], F32)
        nc.sync.dma_start(out=wv_n, in_=wv_d.ap())
        for w_n, off in ((wq_n, 0), (wk_n, C)):
            wps = psw.tile([C, C], F32, tag="work", name="wps")
            nc.tensor.transpose(wps, w_n, ident[0:C, 0:C])
            nc.vector.tensor_copy(out=wstage[0:C, off : off + C], in_=wps)
        nc.sync.dma_start(
            out=wstage[C : C + 1, 0:C], in_=bq_d.ap().unsqueeze(0)
        )
        nc.sync.dma_start(
            out=wstage[C : C + 1, C : 2 * C], in_=bk_d.ap().unsqueeze(0)
        )
        wqk_aug = consts.tile([C1, P], F16)
        nc.vector.tensor_copy(out=wqk_aug, in_=wstage)

        # wv_aug [65, 65] fp16: [0:64,0:64] = Wv^T, row 64 = bv, col 64 = e_64
        vstage = consts.tile([C1, C1], F32)
        nc.gpsimd.memset(vstage, 0.0)
        wvps = psw.tile([C, C], F32, tag="work", name="wvps")
        nc.tensor.transpose(wvps, wv_n, ident[0:C, 0:C])
        nc.vector.tensor_copy(out=vstage[0:C, 0:C], in_=wvps)
        nc.sync.dma_start(
            out=vstage[C : C + 1, 0:C], in_=bv_d.ap().unsqueeze(0)
        )
        nc.vector.memset(vstage[C : C + 1, C : C + 1], 1.0)
        wv_aug = consts.tile([C1, C1], F16)
        nc.vector.tensor_copy(out=wv_aug, in_=vstage)

        x_flat = x_d.ap().rearrange("h t c -> (h t c)")
        out_flat = out_d.ap().rearrange("h t c -> (h t c)")
        n_head = T * C

        def phase1(hh):
            """Generator: load + transpose + projections for global head hh
            (hh counts across REPEAT reps; the x slice is hh % HPC).

            Yields "ready" once the first half's kT and v~ tiles exist —
            head 0 drains to that point, then feeds the rest into its own
            j-loop.
            """
            h = hh % HPC
            x_raw = sb2.tile([P, FREE], F32, name="x_raw")
            nc.sync.dma_start(
                out=x_raw,
                in_=x_flat[h * n_head : (h + 1) * n_head].rearrange(
                    "(p f) -> p f", p=P
                ),
            )
            yield
            x16 = sb2.tile([P, FREE], F16, name="x16")
            xT = sb2.tile([C1, T], F16, name="xT")
            qkT = sb2.tile([P, T], F16, name="qkT")
            kT = sb2.tile([C, T], F16, name="kT")
            vt = sb2.tile([P, NT * C1], F16, name="vt")
            tiles[hh] = (qkT, kT, vt)
            if SKIP_P1:
                yield "ready"
                return
            nc.vector.tensor_copy(out=x16, in_=x_raw)
            nc.gpsimd.memset(xT[C : C + 1, :], 1.0)
            yield

            for half in range(2):
                hsl = slice(half * 1024, half * 1024 + 1024)
                # 4 fp16 transposes of x16 128-col chunks into one psum tile
                pt = psw.tile([P, 512], F16, tag="work", name="pt")
                for b in range(4):
                    s = 4 * half + b
                    nc.tensor.transpose(
                        pt[:, b * P : (b + 1) * P],
                        x16[:, s * P : (s + 1) * P],
                        ident16,
                    )
                    yield
                # batched permuted copies: rows 0:64 -> even g blocks,
                # rows 64:128 -> odd g blocks
                src_lo = pt[0:C, :].rearrange("c (b p) -> c b p", b=4)
                src_hi = pt[C : 2 * C, :].rearrange("c (b p) -> c b p", b=4)
                dst = xT[0:C, hsl].rearrange(
                    "c (b two p) -> c b two p", b=4, two=2
                )
                nc.vector.tensor_copy(out=dst[:, :, 0, :], in_=src_lo)
                nc.vector.tensor_copy(out=dst[:, :, 1, :], in_=src_hi)
                yield
                # qk projection chunks covering these 1024 columns
                for cc in range(2):
                    sl = slice(half * 1024 + cc * 512, half * 1024 + cc * 512 + 512)
                    qkp = psw.tile([P, 512], F32, tag="work", name="qkp")
                    nc.tensor.matmul(qkp, wqk_aug, xT[:, sl], start=True, stop=True)
                    nc.vector.tensor_copy(out=qkT[:, sl], in_=qkp)
                    yield
                # kT for this half (4x-mode SBUF fp16 partition-shift copy)
                nc.vector.tensor_copy(out=kT[:, hsl], in_=qkT[C : 2 * C, hsl])
                yield
                # v~ chunks for the 8 j-tiles of this half
                for vq in range(2):
                    vp = psw.tile([P, 4 * 68], F32, tag="work", name="vp")
                    for b in range(4):
                        g = half * 8 + vq * 4 + b
                        nc.tensor.matmul(
                            vp[:, b * 68 : b * 68 + C1],
                            xT[:, g * P : (g + 1) * P],
                            wv_aug,
                            start=True,
                            stop=True,
                        )
                        if b % 2 == 1:
                            yield
                    g0 = vq * 4 + half * 8
                    nc.vector.tensor_copy(
                        out=vt[:, g0 * C1 : (g0 + 4) * C1].rearrange(
                            "p (b c) -> p b c", b=4
                        ),
                        in_=vp.rearrange("p (b c) -> p b c", b=4)[:, :, 0:C1],
                    )
                    yield
                if half == 0:
                    yield "ready"

        tiles = {}
        from itertools import chain as _chain

        NH = REPEAT * HPC
        if True:
          nxt = phase1(0)
          for step in nxt:
              if step == "ready":
                  break

          for hh in range(NH):
            h = hh % HPC
            qkT, kT, vt = tiles.pop(hh)
            if hh + 1 < NH:
                nxt = _chain(nxt, phase1(hh + 1))

            out_sb = sb2.tile([P, FREE], F32, name="out_sb")

            def phase3(ihalf, av, out_sb=out_sb, h=h):
                if AVT_DMA:
                    avs = sb2.tile([80, IH], F16, name="avs")
                    nc.gpsimd.memset(avs[C:80, :], 0.0)
                    nc.vector.tensor_copy(out=avs[0:C1, :], in_=av)
                    avT = sb2.tile([P, 8 * 80], F16, name="avT")
                    for gg in range(8):
                        nc.sync.dma_start_transpose(
                            avT[:, gg * 80 : gg * 80 + 80],
                            avs[:, gg * P : (gg + 1) * P],
                        )
                else:
                    avs = sb2.tile([C1, IH], F16, name="avs")
                    nc.vector.tensor_copy(out=avs, in_=av)
                    avT = psw.tile([P, 8 * 80], F16, tag="work", name="avT")
                    for gg in range(8):
                        nc.tensor.transpose(
                            avT[:, gg * 80 : gg * 80 + C1],
                            avs[:, gg * P : (gg + 1) * P],
                            ident16[0:C1, 0:C1],
                        )
                rc = sb2.tile([P, 8], F32, name="rc")
                nc.vector.reciprocal(
                    rc, avT.rearrange("p (g c) -> p g c", g=8)[:, :, C : C + 1]
                )
                rc_bc = bass.AP(
                    tensor=rc.tensor,
                    offset=rc.offset,
                    ap=[*rc.ap[:-1], [rc.ap[-1][0], 8], [0, C]],
                )
                nc.vector.tensor_mul(
                    out_sb[:, ihalf * 512 : ihalf * 512 + 512].rearrange(
                        "p (g c) -> p g c", g=8
                    ),
                    avT.rearrange("p (g c) -> p g c", g=8)[:, :, 0:C],
                    rc_bc,
                )
                if ihalf == 1:
                    nc.sync.dma_start(
                        out=out_flat[h * n_head : (h + 1) * n_head].rearrange(
                            "(p f) -> p f", p=P
                        ),
                        in_=out_sb,
                    )

            # ---- phase 2: attention, S/exp pipelined across the i-halves
            av_tiles = {}

            def do_av(ihalf, j, et, vt=vt):
                if SKIP_AV or SKIP_S:
                    if ihalf not in av_tiles:
                        av_tiles[ihalf] = psav.tile(
                            [C1, IH], F32, tag="av", name="av"
                        )
                        nc.vector.memset(av_tiles[ihalf], 1.0)
                else:
                    if ihalf not in av_tiles:
                        av_tiles[ihalf] = psav.tile(
                            [C1, IH], F32, tag="av", name="av"
                        )
                    av = av_tiles[ihalf]
                    for cc in range(2):
                        nc.tensor.matmul(
                            av[:, cc * 512 : (cc + 1) * 512],
                            vt[:, j * C1 : (j + 1) * C1],
                            et[:, cc * 512 : (cc + 1) * 512],
                            start=(j == 0),
                            stop=(j == NT - 1),
                        )
                if j == NT - 1:
                    phase3(ihalf, av_tiles.pop(ihalf))

            pend = deque()
            for chunk in range(2 * NT):
                ihalf, j = divmod(chunk, NT)
                next(nxt, None)
                sp = s_pool.tile([P, IH], F32, tag="sp", name="sp")
                if not SKIP_S:
                    for cc in range(2):
                        nc.tensor.matmul(
                            sp[:, cc * 512 : (cc + 1) * 512],
                            kT[:, j * P : (j + 1) * P],
                            qkT[0:C, ihalf * IH + cc * 512 : ihalf * IH + (cc + 1) * 512],
                            start=True,
                            stop=True,
                        )
                et = etp.tile([P, IH], F16, name="et")
                if not (SKIP_EXP or SKIP_S):
                    if j in DVE_JS:
                        nc.vector.tensor_scalar(
                            out=et.bitcast(U16),
                            in0=sp,
                            scalar1=SCH_A,
                            scalar2=SCH_B,
                            op0=ALU.mult,
                            op1=ALU.add,
                        )
                    else:
                        nc.scalar.activation(
                            et, sp, AF.Exp, scale=0.125, bias=nbias
                        )
                elif not (SKIP_AV or SKIP_S):
                    nc.vector.memset(et[:, 0:1], 1.0)
                if len(pend) >= AV_LAG:
                    pend.popleft()()
                pend.append(lambda ihalf=ihalf, j=j, et=et: do_av(ihalf, j, et))
            while pend:
                pend.popleft()()

            # finish emitting the next head's phase 1 before its j-loop
            for _ in nxt:
                pass
            nxt = iter(())
    finally:
        for cm in reversed(ctxs):
            cm.__exit__(None, None, None)


_NC_CACHE = {}


def build_nc():
    if "nc" in _NC_CACHE:
        return _NC_CACHE["nc"]
    nc = bacc.Bacc(
        "TRN2", target_bir_lowering=False, debug=False, num_devices=N_CORES
    )
    x_d = nc.dram_tensor("x", [HPC, T, C], F32, kind="ExternalInput")
    wq_d = nc.dram_tensor("Wq", [C, C], F32, kind="ExternalInput")
    bq_d = nc.dram_tensor("bq", [C], F32, kind="ExternalInput")
    wk_d = nc.dram_tensor("Wk", [C, C], F32, kind="ExternalInput")
    bk_d = nc.dram_tensor("bk", [C], F32, kind="ExternalInput")
    wv_d = nc.dram_tensor("Wv", [C, C], F32, kind="ExternalInput")
    bv_d = nc.dram_tensor("bv", [C], F32, kind="ExternalInput")
    out_d = nc.dram_tensor("out", [HPC, T, C], F32, kind="ExternalOutput")

    with tile.TileContext(nc) as tc:
        _build_tile_kernel(
            tc, nc, x_d, wq_d, bq_d, wk_d, bk_d, wv_d, bv_d, out_d
        )
    nc.compile()
    _NC_CACHE["nc"] = nc
    return nc


def _get_exec():
    """Build the sharded jitted executable once and cache it."""
    if "exec" in _NC_CACHE:
        return _NC_CACHE["exec"]
    import jax
    from jax.sharding import Mesh, PartitionSpec
    from jax.experimental.shard_map import shard_map
    from concourse import bass2jax

    nc = build_nc()
    bass2jax.install_neuronx_cc_hook()

    in_names, out_names, out_avals, zero_outs = [], [], [], []
    partition_name = (
        nc.partition_id_tensor.name if nc.partition_id_tensor else None
    )
    for alloc in nc.m.functions[0].allocations:
        if not isinstance(alloc, mybir.MemoryLocationSet):
            continue
        name = alloc.memorylocations[0].name
        if alloc.kind == "ExternalInput":
            if name != partition_name:
                in_names.append(name)
        elif alloc.kind == "ExternalOutput":
            out_avals.append(
                jax.core.ShapedArray(
                    tuple(alloc.tensor_shape), mybir.dt.np(alloc.dtype)
                )
            )
            zero_outs.append(
                np.zeros(tuple(alloc.tensor_shape), mybir.dt.np(alloc.dtype))
            )
            out_names.append(name)

    n_params = len(in_names)
    in_names.extend(out_names)
    if partition_name is not None:
        in_names.append(partition_name)

    def _body(*args):
        operands = list(args)
        if partition_name is not None:
            operands.append(bass2jax.partition_id_tensor())
        outs = bass2jax._bass_exec_p.bind(
            *operands,
            out_avals=tuple(out_avals),
            in_names=tuple(in_names),
            out_names=tuple(out_names),
            lowering_input_output_aliases=(),
            sim_require_finite=True,
            sim_require_nnan=True,
            nc=nc,
        )
        return tuple(outs)

    devices = jax.devices()[:N_CORES]
    mesh = Mesh(np.asarray(devices), ("core",))
    n_outs = len(out_names)
    sharded = jax.jit(
        shard_map(
            _body,
            mesh=mesh,
            in_specs=(PartitionSpec("core"),) * (n_params + n_outs),
            out_specs=(PartitionSpec("core"),) * n_outs,
            check_rep=False,
        ),
        keep_unused=True,
    )
    cz = [
        np.zeros((N_CORES * z.shape[0], *z.shape[1:]), z.dtype)
        for z in zero_outs
    ]
    _NC_CACHE["exec"] = (sharded, in_names[:n_params], out_names, cz)
    return _NC_CACHE["exec"]


def _concat_inputs(x, Wq, bq, Wk, bk, Wv, bv, in_names):
    """Global (concatenated along axis 0) input arrays, in NEFF input order."""
    xf = np.ascontiguousarray(np.asarray(x, dtype=np.float32)).reshape(
        HEADS, T, C
    )
    per = {
        "x": xf,
        "Wq": np.tile(np.asarray(Wq, np.float32), (N_CORES, 1)),
        "bq": np.tile(np.asarray(bq, np.float32), N_CORES),
        "Wk": np.tile(np.asarray(Wk, np.float32), (N_CORES, 1)),
        "bk": np.tile(np.asarray(bk, np.float32), N_CORES),
        "Wv": np.tile(np.asarray(Wv, np.float32), (N_CORES, 1)),
        "bv": np.tile(np.asarray(bv, np.float32), N_CORES),
    }
    return [per[name] for name in in_names]


def kernel(x, Wq, bq, Wk, bk, Wv, bv):
    try:
        sharded, in_names, out_names, cz = _get_exec()
        ins = _concat_inputs(x, Wq, bq, Wk, bk, Wv, bv, in_names)
        out_arrs = sharded(*ins, *cz)
        out = np.asarray(out_arrs[out_names.index("out")])
        return out.reshape(B, K, T, C).astype(np.float32, copy=False)
    except Exception:
        nc = build_nc()
        xf = np.ascontiguousarray(np.asarray(x, np.float32)).reshape(
            HEADS, T, C
        )
        weights = {
            "Wq": np.ascontiguousarray(np.asarray(Wq, np.float32)),
            "bq": np.ascontiguousarray(np.asarray(bq, np.float32)),
            "Wk": np.ascontiguousarray(np.asarray(Wk, np.float32)),
            "bk": np.ascontiguousarray(np.asarray(bk, np.float32)),
            "Wv": np.ascontiguousarray(np.asarray(Wv, np.float32)),
            "bv": np.ascontiguousarray(np.asarray(bv, np.float32)),
        }
        in_maps = [
            {"x": np.ascontiguousarray(xf[c * HPC : (c + 1) * HPC]), **weights}
            for c in range(N_CORES)
        ]
        res = run_bass_kernel_spmd(nc, in_maps, list(range(N_CORES))).results
        out = np.concatenate([res[c]["out"] for c in range(N_CORES)], axis=0)
        return out.reshape(B, K, T, C).astype(np.float32, copy=False)


def time_hw(inputs_np, lo=16, hi=128):
    """Estimate true on-device time per workload via the R-repeat slope."""
    import time as _time
    import jax

    global REPEAT

    def build_at(r):
        global REPEAT
        old = REPEAT
        REPEAT = r
        _NC_CACHE.clear()
        try:
            sharded, in_names, out_names, cz = _get_exec()
            ins = _concat_inputs(
                inputs_np["x"], inputs_np["Wq"], inputs_np["bq"],
                inputs_np["Wk"], inputs_np["bk"], inputs_np["Wv"],
                inputs_np["bv"], in_names,
            )
            dev_args = [jax.device_put(a) for a in ins + cz]
            jax.block_until_ready(sharded(*dev_args))
            return sharded, dev_args
        finally:
            REPEAT = old
            _NC_CACHE.clear()

    f_lo, a_lo = build_at(lo)
    f_hi, a_hi = build_at(hi)

    def batch(f, a, iters=8):
        t0 = _time.perf_counter()
        o = None
        for _ in range(iters):
            o = f(*a)
        jax.block_until_ready(o)
        return (_time.perf_counter() - t0) / iters

    t_lo, t_hi = [], []
    for _ in range(12):
        t_lo.append(batch(f_lo, a_lo))
        t_hi.append(batch(f_hi, a_hi))
    return (min(t_hi) - min(t_lo)) / (hi - lo) * 1e9


if __name__ == "__main__":
    rng = np.random.default_rng(0)
    ins = {
        "x": rng.standard_normal((B, K, T, C), dtype=np.float32),
        "Wq": rng.standard_normal((C, C), dtype=np.float32) / 8,
        "bq": rng.standard_normal((C,), dtype=np.float32) * 0.01,
        "Wk": rng.standard_normal((C, C), dtype=np.float32) / 8,
        "bk": rng.standard_normal((C,), dtype=np.float32) * 0.01,
        "Wv": rng.standard_normal((C, C), dtype=np.float32) / 8,
        "bv": rng.standard_normal((C,), dtype=np.float32) * 0.01,
    }
    out = kernel(**ins)
    print(out.shape, out.dtype)


# revision 36
# speedup vs baseline: 1.1831x; 1.1831x over previous
"""Trainium2 Bass kernel for CDimSelfAttention (v2).

Problem: x [B=4, K=8, T=2048, C=64] f32; per (b,k) head:
  q = x @ Wq.T + bq ; k = x @ Wk.T + bk ; v = x @ Wv.T + bv
  out = softmax(q k^T / sqrt(C)) v

Sharding: data-parallel over flattened (b,k) — 32 heads, 4 per core on
8 cores. Weights replicated.

v2 design (vs v1 baseline at ~249us): instruction-count and engine-balance
rework, keeping v1's permuted-column trick (column u = g*128 + p <-> row
t = 16p + g, softmax/AV permutation-invariant, un-permuted for free at the
final store).

  - x [128,1024] f32 -> fp16 once (DVE), fp16 PE transposes (1 cyc/col),
    batched partition-shifted DVE copies into xT_aug [65, 2048] whose row 64
    is constant 1.0 (ones-row).
  - Biases folded into the matmuls via the ones-row: one stationary
    wqk_aug [65, 128] computes q (psum rows 0:64) AND k (rows 64:128) with
    biases, in 4x 512-col matmuls -> one fp16 copy -> qkT; kT extracted by a
    cheap 4x-mode SBUF fp16 copy (partition shift).
  - v~ [128, 16*65] fp16 via 16 matmuls vs wv_aug [65, 65] (bias row + ones
    column folded), 4 batched psum->fp16 converts.
  - exp(s - 1.7) (global shift, cancels in softmax; keeps exp < fp16 range):
    ACT activation for 13 of 16 j-tiles; DVE computes the other 3 via a
    one-op Schraudolph: uint16 bits = s*(log2e*128) + const -> bitcast fp16.
    (measured HW: total rel err 8.3e-3 vs 2e-2 budget)
  - AV accumulated in psum [65, 1024] per i-half (v1 scheme), ones column
    gives row sums.
  - Finalize: psav -> fp16 avs, fp16 PE transposes into one psum tile
    [128, 8*80], one strided reciprocal (row sums), one broadcast
    tensor_mul -> out_sb; single DMA store per head.

Software pipelining: the (ihalf, j) chunk loop is flattened so S/exp
pipeline across the i-half boundary (only AV waits on the psav barrier,
absorbed by the AV_LAG emission lag); phase 1 of head h+1 is interleaved
into head h's chunk loop as generator steps, and head 0 starts its own
loop as soon as the first half of its phase 1 is emitted.

Measured on HW: v1 baseline 249.5us -> v2 240.5us. Both runs are bound by
the PE instruction stream (~1363 PE instructions x ~175ns dispatch/sem
overhead each); engine busy times (sim): PE 119us, ACT 109us, DVE 73us.
The matmul count is pinned by the PSUM-bank limit (a single matmul output
cannot cross a 2KB bank boundary -> max 512 fp32 columns), so S and AV
need 2 matmuls per [128, 1024] tile each, x32 tiles x4 heads, plus one
implicit Ldweights per matmul emitted by the tile scheduler.
"""

from collections import deque

import numpy as np

import concourse.bass as bass
import concourse.mybir as mybir
import concourse.tile as tile
from concourse import bacc
from concourse.bass_utils import run_bass_kernel_spmd
from concourse.masks import make_identity

F32 = mybir.dt.float32
F16 = mybir.dt.float16
U16 = mybir.dt.uint16
AF = mybir.ActivationFunctionType
ALU = mybir.AluOpType

B, K, T, C = 4, 8, 2048, 64
N_CORES = 8
HEADS = B * K            # 32
HPC = HEADS // N_CORES   # 4 heads per core
P = 128                  # partitions
NT = T // P              # 16 t-tiles / j-tiles
FREE = T * C // P        # 1024 free elems of one head slice on 128 partitions
C1 = C + 1               # 65: v plus ones column
IH = T // 2              # 1024, i-half size
REPEAT = 1               # repeat whole per-core workload (timing experiments)
SKIP_EXP = False
SKIP_AV = False
SKIP_S = False
SKIP_P1 = False
AV_LAG = 4               # AV matmul emission lag (steps)
AVT_DMA = False          # finalize transposes via DMA xbar instead of PE
ET_BUFS = 6
EXP_SHIFT = 1.7          # exp(s - shift): keeps exp(s) <= e^5.6 ~ 270
DVE_JS = (0, 3, 5, 8, 11, 14)  # j-tiles whose exp runs on DVE (Schraudolph)
LOG2E = 1.4426950408889634
SCH_MAGIC = -59.0        # Schraudolph centering (fp16-bits units)
# uint16 fp16-bits = s_raw * SCH_A + SCH_B  (s_raw = q.k before /8 scaling)
SCH_A = 0.125 * LOG2E * 1024.0
SCH_B = 1024.0 * (15.0 - EXP_SHIFT * LOG2E) + SCH_MAGIC + 0.5  # +0.5: trunc->round


def _build_tile_kernel(tc, nc, x_d, wq_d, bq_d, wk_d, bk_d, wv_d, bv_d, out_d):
    ctxs = []

    def pool(**kw):
        cm = tc.tile_pool(**kw)
        p = cm.__enter__()
        ctxs.append(cm)
        return p

    try:
        consts = pool(name="consts", bufs=1)
        sb2 = pool(name="sb2", bufs=2)
        etp = pool(name="etp", bufs=ET_BUFS)
        # PSUM: 8 banks of 2KB/partition.
        #   s_pool: S^T matmul tiles [128,1024]f32 = 2 banks x 2 bufs = 4
        #   psav:   A@V accumulator  [65,1024]f32  = 2 banks x 1 buf  = 2
        #   psw:    work tiles (<=1 bank each)     = 1 bank x 2 bufs  = 2
        s_pool = pool(name="s_pool", bufs=2, space="PSUM")
        psw = pool(name="psw", bufs=2, space="PSUM")
        psav = pool(name="psav", bufs=1, space="PSUM")

        # ---- constants ----
        ident = consts.tile([P, P], F32)
        make_identity(nc, ident)
        ident16 = consts.tile([P, P], F16)
        nc.vector.tensor_copy(out=ident16, in_=ident)
        nbias = consts.tile([P, 1], F32)
        nc.gpsimd.memset(nbias, -EXP_SHIFT)

        # wqk_aug [65, 128] fp16: rows 0:64 = Wq^T | Wk^T, row 64 = bq | bk
        wstage = consts.tile([C1, P], F32)
        wq_n = consts.tile([C, C], F32)
        nc.sync.dma_start(out=wq_n, in_=wq_d.ap())
        wk_n = consts.tile([C, C], F32)
        nc.sync.dma_start(out=wk_n, in_=wk_d.ap())
        wv_n = consts.tile([C, C], F32)
        nc.sync.dma_start(out=wv_n, in_=wv_d.ap())
        for w_n, off in ((wq_n, 0), (wk_n, C)):
            wps = psw.tile([C, C], F32, tag="work", name="wps")
            nc.tensor.transpose(wps, w_n, ident[0:C, 0:C])
            nc.vector.tensor_copy(out=wstage[0:C, off : off + C], in_=wps)
        nc.sync.dma_start(
            out=wstage[C : C + 1, 0:C], in_=bq_d.ap().unsqueeze(0)
        )
        nc.sync.dma_start(
            out=wstage[C : C + 1, C : 2 * C], in_=bk_d.ap().unsqueeze(0)
        )
        wqk_aug = consts.tile([C1, P], F16)
        nc.vector.tensor_copy(out=wqk_aug, in_=wstage)

        # wv_aug [65, 65] fp16: [0:64,0:64] = Wv^T, row 64 = bv, col 64 = e_64
        vstage = consts.tile([C1, C1], F32)
        nc.gpsimd.memset(vstage, 0.0)
        wvps = psw.tile([C, C], F32, tag="work", name="wvps")
        nc.tensor.transpose(wvps, wv_n, ident[0:C, 0:C])
        nc.vector.tensor_copy(out=vstage[0:C, 0:C], in_=wvps)
        nc.sync.dma_start(
            out=vstage[C : C + 1, 0:C], in_=bv_d.ap().unsqueeze(0)
        )
        nc.vector.memset(vstage[C : C + 1, C : C + 1], 1.0)
        wv_aug = consts.tile([C1, C1], F16)
        nc.vector.tensor_copy(out=wv_aug, in_=vstage)

        x_flat = x_d.ap().rearrange("h t c -> (h t c)")
        out_flat = out_d.ap().rearrange("h t c -> (h t c)")
        n_head = T * C

        def phase1(hh):
            """Generator: load + transpose + projections for global head hh
            (hh counts across REPEAT reps; the x slice is hh % HPC).

            Yields "ready" once the first half's kT and v~ tiles exist —
            head 0 drains to that point, then feeds the rest into its own
            j-loop.
            """
            h = hh % HPC
            x_raw = sb2.tile([P, FREE], F32, name="x_raw")
            nc.sync.dma_start(
                out=x_raw,
                in_=x_flat[h * n_head : (h + 1) * n_head].rearrange(
                    "(p f) -> p f", p=P
                ),
            )
            yield
            x16 = sb2.tile([P, FREE], F16, name="x16")
            xT = sb2.tile([C1, T], F16, name="xT")
            qkT = sb2.tile([P, T], F16, name="qkT")
            kT = sb2.tile([C, T], F16, name="kT")
            vt = sb2.tile([P, NT * C1], F16, name="vt")
            tiles[hh] = (qkT, kT, vt)
            if SKIP_P1:
                yield "ready"
                return
            nc.vector.tensor_copy(out=x16, in_=x_raw)
            nc.gpsimd.memset(xT[C : C + 1, :], 1.0)
            yield

            for half in range(2):
                hsl = slice(half * 1024, half * 1024 + 1024)
                # 4 fp16 transposes of x16 128-col chunks into one psum tile
                pt = psw.tile([P, 512], F16, tag="work", name="pt")
                for b in range(4):
                    s = 4 * half + b
                    nc.tensor.transpose(
                        pt[:, b * P : (b + 1) * P],
                        x16[:, s * P : (s + 1) * P],
                        ident16,
                    )
                    yield
                # batched permuted copies: rows 0:64 -> even g blocks,
                # rows 64:128 -> odd g blocks
                src_lo = pt[0:C, :].rearrange("c (b p) -> c b p", b=4)
                src_hi = pt[C : 2 * C, :].rearrange("c (b p) -> c b p", b=4)
                dst = xT[0:C, hsl].rearrange(
                    "c (b two p) -> c b two p", b=4, two=2
                )
                nc.vector.tensor_copy(out=dst[:, :, 0, :], in_=src_lo)
                nc.vector.tensor_copy(out=dst[:, :, 1, :], in_=src_hi)
                yield
                # qk projection chunks covering these 1024 columns
                for cc in range(2):
                    sl = slice(half * 1024 + cc * 512, half * 1024 + cc * 512 + 512)
                    qkp = psw.tile([P, 512], F32, tag="work", name="qkp")
                    nc.tensor.matmul(qkp, wqk_aug, xT[:, sl], start=True, stop=True)
                    nc.vector.tensor_copy(out=qkT[:, sl], in_=qkp)
                    yield
                # kT for this half (4x-mode SBUF fp16 partition-shift copy)
                nc.vector.tensor_copy(out=kT[:, hsl], in_=qkT[C : 2 * C, hsl])
                yield
                # v~ chunks for the 8 j-tiles of this half
                for vq in range(2):
                    vp = psw.tile([P, 4 * 68], F32, tag="work", name="vp")
                    for b in range(4):
                        g = half * 8 + vq * 4 + b
                        nc.tensor.matmul(
                            vp[:, b * 68 : b * 68 + C1],
                            xT[:, g * P : (g + 1) * P],
                            wv_aug,
                            start=True,
                            stop=True,
                        )
                        if b % 2 == 1:
                            yield
                    g0 = vq * 4 + half * 8
                    nc.vector.tensor_copy(
                        out=vt[:, g0 * C1 : (g0 + 4) * C1].rearrange(
                            "p (b c) -> p b c", b=4
                        ),
                        in_=vp.rearrange("p (b c) -> p b c", b=4)[:, :, 0:C1],
                    )
                    yield
                if half == 0:
                    yield "ready"

        tiles = {}
        from itertools import chain as _chain

        NH = REPEAT * HPC
        if True:
          nxt = phase1(0)
          for step in nxt:
              if step == "ready":
                  break

          for hh in range(NH):
            h = hh % HPC
            qkT, kT, vt = tiles.pop(hh)
            if hh + 1 < NH:
                nxt = _chain(nxt, phase1(hh + 1))

            out_sb = sb2.tile([P, FREE], F32, name="out_sb")

            def phase3(ihalf, av, out_sb=out_sb, h=h):
                if AVT_DMA:
                    avs = sb2.tile([80, IH], F16, name="avs")
                    nc.gpsimd.memset(avs[C:80, :], 0.0)
                    nc.vector.tensor_copy(out=avs[0:C1, :], in_=av)
                    avT = sb2.tile([P, 8 * 80], F16, name="avT")
                    for gg in range(8):
                        nc.sync.dma_start_transpose(
                            avT[:, gg * 80 : gg * 80 + 80],
                            avs[:, gg * P : (gg + 1) * P],
                        )
                else:
                    avs = sb2.tile([C1, IH], F16, name="avs")
                    nc.vector.tensor_copy(out=avs, in_=av)
                    avT = psw.tile([P, 8 * 80], F16, tag="work", name="avT")
                    for gg in range(8):
                        nc.tensor.transpose(
                            avT[:, gg * 80 : gg * 80 + C1],
                            avs[:, gg * P : (gg + 1) * P],
                            ident16[0:C1, 0:C1],
                        )
                rc = sb2.tile([P, 8], F32, name="rc")
                nc.vector.reciprocal(
                    rc, avT.rearrange("p (g c) -> p g c", g=8)[:, :, C : C + 1]
                )
                rc_bc = bass.AP(
                    tensor=rc.tensor,
                    offset=rc.offset,
                    ap=[*rc.ap[:-1], [rc.ap[-1][0], 8], [0, C]],
                )
                nc.vector.tensor_mul(
                    out_sb[:, ihalf * 512 : ihalf * 512 + 512].rearrange(
                        "p (g c) -> p g c", g=8
                    ),
                    avT.rearrange("p (g c) -> p g c", g=8)[:, :, 0:C],
                    rc_bc,
                )
                if ihalf == 1:
                    nc.sync.dma_start(
                        out=out_flat[h * n_head : (h + 1) * n_head].rearrange(
                            "(p f) -> p f", p=P
                        ),
                        in_=out_sb,
                    )

            # ---- phase 2: attention, S/exp pipelined across the i-halves
            av_tiles = {}

            def do_av(ihalf, j, et, vt=vt):
                if SKIP_AV or SKIP_S:
                    if ihalf not in av_tiles:
                        av_tiles[ihalf] = psav.tile(
                            [C1, IH], F32, tag="av", name="av"
                        )
                        nc.vector.memset(av_tiles[ihalf], 1.0)
                else:
                    if ihalf not in av_tiles:
                        av_tiles[ihalf] = psav.tile(
                            [C1, IH], F32, tag="av", name="av"
                        )
                    av = av_tiles[ihalf]
                    for cc in range(2):
                        nc.tensor.matmul(
                            av[:, cc * 512 : (cc + 1) * 512],
                            vt[:, j * C1 : (j + 1) * C1],
                            et[:, cc * 512 : (cc + 1) * 512],
                            start=(j == 0),
                            stop=(j == NT - 1),
                        )
                if j == NT - 1:
                    phase3(ihalf, av_tiles.pop(ihalf))

            pend = deque()
            for chunk in range(2 * NT):
                ihalf, j = divmod(chunk, NT)
                next(nxt, None)
                sp = s_pool.tile([P, IH], F32, tag="sp", name="sp")
                if not SKIP_S:
                    for cc in range(2):
                        nc.tensor.matmul(
                            sp[:, cc * 512 : (cc + 1) * 512],
                            kT[:, j * P : (j + 1) * P],
                            qkT[0:C, ihalf * IH + cc * 512 : ihalf * IH + (cc + 1) * 512],
                            start=True,
                            stop=True,
                        )
                et = etp.tile([P, IH], F16, name="et")
                if not (SKIP_EXP or SKIP_S):
                    if j in DVE_JS:
                        nc.vector.tensor_scalar(
                            out=et.bitcast(U16),
                            in0=sp,
                            scalar1=SCH_A,
                            scalar2=SCH_B,
                            op0=ALU.mult,
                            op1=ALU.add,
                        )
                    else:
                        nc.scalar.activation(
                            et, sp, AF.Exp, scale=0.125, bias=nbias
                        )
                elif not (SKIP_AV or SKIP_S):
                    nc.vector.memset(et[:, 0:1], 1.0)
                if len(pend) >= AV_LAG:
                    pend.popleft()()
                pend.append(lambda ihalf=ihalf, j=j, et=et: do_av(ihalf, j, et))
            while pend:
                pend.popleft()()

            # finish emitting the next head's phase 1 before its j-loop
            for _ in nxt:
                pass
            nxt = iter(())
    finally:
        for cm in reversed(ctxs):
            cm.__exit__(None, None, None)


_NC_CACHE = {}


def build_nc():
    if "nc" in _NC_CACHE:
        return _NC_CACHE["nc"]
    nc = bacc.Bacc(
        "TRN2", target_bir_lowering=False, debug=False, num_devices=N_CORES
    )
    x_d = nc.dram_tensor("x", [HPC, T, C], F32, kind="ExternalInput")
    wq_d = nc.dram_tensor("Wq", [C, C], F32, kind="ExternalInput")
    bq_d = nc.dram_tensor("bq", [C], F32, kind="ExternalInput")
    wk_d = nc.dram_tensor("Wk", [C, C], F32, kind="ExternalInput")
    bk_d = nc.dram_tensor("bk", [C], F32, kind="ExternalInput")
    wv_d = nc.dram_tensor("Wv", [C, C], F32, kind="ExternalInput")
    bv_d = nc.dram_tensor("bv", [C], F32, kind="ExternalInput")
    out_d = nc.dram_tensor("out", [HPC, T, C], F32, kind="ExternalOutput")

    with tile.TileContext(nc) as tc:
        _build_tile_kernel(
            tc, nc, x_d, wq_d, bq_d, wk_d, bk_d, wv_d, bv_d, out_d
        )
    nc.compile()
    _NC_CACHE["nc"] = nc
    return nc


def _get_exec():
    """Build the sharded jitted executable once and cache it."""
    if "exec" in _NC_CACHE:
        return _NC_CACHE["exec"]
    import jax
    from jax.sharding import Mesh, PartitionSpec
    from jax.experimental.shard_map import shard_map
    from concourse import bass2jax

    nc = build_nc()
    bass2jax.install_neuronx_cc_hook()

    in_names, out_names, out_avals, zero_outs = [], [], [], []
    partition_name = (
        nc.partition_id_tensor.name if nc.partition_id_tensor else None
    )
    for alloc in nc.m.functions[0].allocations:
        if not isinstance(alloc, mybir.MemoryLocationSet):
            continue
        name = alloc.memorylocations[0].name
        if alloc.kind == "ExternalInput":
            if name != partition_name:
                in_names.append(name)
        elif alloc.kind == "ExternalOutput":
            out_avals.append(
                jax.core.ShapedArray(
                    tuple(alloc.tensor_shape), mybir.dt.np(alloc.dtype)
                )
            )
            zero_outs.append(
                np.zeros(tuple(alloc.tensor_shape), mybir.dt.np(alloc.dtype))
            )
            out_names.append(name)

    n_params = len(in_names)
    in_names.extend(out_names)
    if partition_name is not None:
        in_names.append(partition_name)

    def _body(*args):
        operands = list(args)
        if partition_name is not None:
            operands.append(bass2jax.partition_id_tensor())
        outs = bass2jax._bass_exec_p.bind(
            *operands,
            out_avals=tuple(out_avals),
            in_names=tuple(in_names),
            out_names=tuple(out_names),
            lowering_input_output_aliases=(),
            sim_require_finite=True,
            sim_require_nnan=True,
            nc=nc,
        )
        return tuple(outs)

    devices = jax.devices()[:N_CORES]
    mesh = Mesh(np.asarray(devices), ("core",))
    n_outs = len(out_names)
    sharded = jax.jit(
        shard_map(
            _body,
            mesh=mesh,
            in_specs=(PartitionSpec("core"),) * (n_params + n_outs),
            out_specs=(PartitionSpec("core"),) * n_outs,
            check_rep=False,
        ),
        keep_unused=True,
    )
    cz = [
        np.zeros((N_CORES * z.shape[0], *z.shape[1:]), z.dtype)
        for z in zero_outs
    ]
    _NC_CACHE["exec"] = (sharded, in_names[:n_params], out_names, cz)
    return _NC_CACHE["exec"]


def _concat_inputs(x, Wq, bq, Wk, bk, Wv, bv, in_names):
    """Global (concatenated along axis 0) input arrays, in NEFF input order."""
    xf = np.ascontiguousarray(np.asarray(x, dtype=np.float32)).reshape(
        HEADS, T, C
    )
    per = {
        "x": xf,
        "Wq": np.tile(np.asarray(Wq, np.float32), (N_CORES, 1)),
        "bq": np.tile(np.asarray(bq, np.float32), N_CORES),
        "Wk": np.tile(np.asarray(Wk, np.float32), (N_CORES, 1)),
        "bk": np.tile(np.asarray(bk, np.float32), N_CORES),
        "Wv": np.tile(np.asarray(Wv, np.float32), (N_CORES, 1)),
        "bv": np.tile(np.asarray(bv, np.float32), N_CORES),
    }
    return [per[name] for name in in_names]


def kernel(x, Wq, bq, Wk, bk, Wv, bv):
    try:
        sharded, in_names, out_names, cz = _get_exec()
        ins = _concat_inputs(x, Wq, bq, Wk, bk, Wv, bv, in_names)
        out_arrs = sharded(*ins, *cz)
        out = np.asarray(out_arrs[out_names.index("out")])
        return out.reshape(B, K, T, C).astype(np.float32, copy=False)
    except Exception:
        nc = build_nc()
        xf = np.ascontiguousarray(np.asarray(x, np.float32)).reshape(
            HEADS, T, C
        )
        weights = {
            "Wq": np.ascontiguousarray(np.asarray(Wq, np.float32)),
            "bq": np.ascontiguousarray(np.asarray(bq, np.float32)),
            "Wk": np.ascontiguousarray(np.asarray(Wk, np.float32)),
            "bk": np.ascontiguousarray(np.asarray(bk, np.float32)),
            "Wv": np.ascontiguousarray(np.asarray(Wv, np.float32)),
            "bv": np.ascontiguousarray(np.asarray(bv, np.float32)),
        }
        in_maps = [
            {"x": np.ascontiguousarray(xf[c * HPC : (c + 1) * HPC]), **weights}
            for c in range(N_CORES)
        ]
        res = run_bass_kernel_spmd(nc, in_maps, list(range(N_CORES))).results
        out = np.concatenate([res[c]["out"] for c in range(N_CORES)], axis=0)
        return out.reshape(B, K, T, C).astype(np.float32, copy=False)


def time_hw(inputs_np, lo=16, hi=128):
    """Estimate true on-device time per workload via the R-repeat slope."""
    import time as _time
    import jax

    global REPEAT

    def build_at(r):
        global REPEAT
        old = REPEAT
        REPEAT = r
        _NC_CACHE.clear()
        try:
            sharded, in_names, out_names, cz = _get_exec()
            ins = _concat_inputs(
                inputs_np["x"], inputs_np["Wq"], inputs_np["bq"],
                inputs_np["Wk"], inputs_np["bk"], inputs_np["Wv"],
                inputs_np["bv"], in_names,
            )
            dev_args = [jax.device_put(a) for a in ins + cz]
            jax.block_until_ready(sharded(*dev_args))
            return sharded, dev_args
        finally:
            REPEAT = old
            _NC_CACHE.clear()

    f_lo, a_lo = build_at(lo)
    f_hi, a_hi = build_at(hi)

    def batch(f, a, iters=8):
        t0 = _time.perf_counter()
        o = None
        for _ in range(iters):
            o = f(*a)
        jax.block_until_ready(o)
        return (_time.perf_counter() - t0) / iters

    t_lo, t_hi = [], []
    for _ in range(12):
        t_lo.append(batch(f_lo, a_lo))
        t_hi.append(batch(f_hi, a_hi))
    return (min(t_hi) - min(t_lo)) / (hi - lo) * 1e9


if __name__ == "__main__":
    rng = np.random.default_rng(0)
    ins = {
        "x": rng.standard_normal((B, K, T, C), dtype=np.float32),
        "Wq": rng.standard_normal((C, C), dtype=np.float32) / 8,
        "bq": rng.standard_normal((C,), dtype=np.float32) * 0.01,
        "Wk": rng.standard_normal((C, C), dtype=np.float32) / 8,
        "bk": rng.standard_normal((C,), dtype=np.float32) * 0.01,
        "Wv": rng.standard_normal((C, C), dtype=np.float32) / 8,
        "bv": rng.standard_normal((C,), dtype=np.float32) * 0.01,
    }
    out = kernel(**ins)
    print(out.shape, out.dtype)
